# revision 1
# baseline (speedup 1.0000x reference)
"""DenseGGNN (gnn_message_passing) Trainium2 Bass kernel.

Math per layer i (per batch):
    s  = A^T @ h                    # [N, C], A binary adjacency
    gx = s @ (W_i @ w_ih_i^T)       # fused:  ((A^T h) W) @ w_ih^T
    gh = h @ w_hh_i^T
    r  = sigmoid(gx_r + gh_r + b_r);  zc = 1 - z = sigmoid(-(gx_z + gh_z + b_z))
    n  = tanh(gx_n + b_in + r * (gh_n + b_hn))
    h' = h + zc * (n - h)

Device layout ("T-layout"): state hT is feature-major [C=128 part, N=1024].
The s-matmul consumes h in node-major fp16 hi/lo tiles (split precision:
h = h_hi + h_lo, each fp16; the adjacency is exact in fp16, so s gets
~fp32 accuracy from two matmul passes).  gx uses split-fp16 for both s and
the fused weight; gh runs single-pass fp16 (|gh| << |gx| so its error
contribution is small).  PSUM accumulates in fp32.  Layers >= RELAX_FROM
drop the h_lo and s_lo correction passes: perturbations introduced late
amplify far less through the remaining GRU layers (measured 1.2e-3 final
rel err vs 3.2e-4 fully split).

All layout changes ride the DMA xbar transpose (fp16) — the PE does only
matmuls.  Sharding: batch (32) split across 8 cores, 4 batches/core,
weights replicated; no cross-core communication.
"""

from contextlib import ExitStack, nullcontext

import numpy as np

import concourse.bass as bass
import concourse.bacc as bacc
import concourse.tile as tile
import concourse.mybir as mybir
from concourse.bass_utils import run_bass_kernel_spmd
from concourse.masks import make_identity

B, N, C, L = 32, 1024, 128, 4
NCORES = 8
BPC = B // NCORES          # batches per core
P = 128                    # partitions
NT = N // P                # node tiles (8)
HALF = 512                 # psum-bank-sized column chunk
S_RF = 2                   # layers < this get the h_lo pass through A^T

F32 = mybir.dt.float32
F16 = mybir.dt.float16
AF = mybir.ActivationFunctionType
ALU = mybir.AluOpType

_PROGRAM_CACHE = {}


def _build_program(reps: int = 1, loop_reps: int = 1) -> bass.Bass:
    # reps > 1 re-emits the whole body back-to-back in one NEFF;
    # loop_reps > 1 wraps the body in a hardware For_i loop.  Both are
    # benchmarking aids (wall-time slope isolates per-iteration device
    # time from the axon dispatch overhead).
    nc = bacc.Bacc()

    x_d = nc.declare_dram_parameter("x", [BPC, N, C], F32, isOutput=False)
    adj_d = nc.declare_dram_parameter("adj", [BPC, N, N], F16, isOutput=False)
    wch_d = nc.declare_dram_parameter("wch", [C, L, 3, C], F16, isOutput=False)
    wcl_d = nc.declare_dram_parameter("wcl", [C, L, 3, C], F16, isOutput=False)
    whh_d = nc.declare_dram_parameter("whh", [C, L, 3, C], F16, isOutput=False)
    bias_d = nc.declare_dram_parameter("bias", [C, L, 4], F32, isOutput=False)
    y_d = nc.declare_dram_parameter("y", [BPC, N, C], F32, isOutput=True)

    with tile.TileContext(nc) as tc, ExitStack() as ctx:
        consts = ctx.enter_context(tc.tile_pool(name="consts", bufs=1))
        adj_pool = ctx.enter_context(tc.tile_pool(name="adjp", bufs=1))
        xo_pool = ctx.enter_context(tc.tile_pool(name="xo", bufs=1))
        hnm_pool = ctx.enter_context(tc.tile_pool(name="hnm", bufs=1))
        hT_pool = ctx.enter_context(tc.tile_pool(name="hT", bufs=2))
        hTh_pool = ctx.enter_context(tc.tile_pool(name="hTh", bufs=2))
        hTl_pool = ctx.enter_context(tc.tile_pool(name="hTl", bufs=1))
        sT_pool = ctx.enter_context(tc.tile_pool(name="sT", bufs=2))
        ew_pool = ctx.enter_context(tc.tile_pool(name="ew", bufs=10))
        ps_s = ctx.enter_context(tc.tile_pool(name="ps_s", bufs=2, space="PSUM"))
        ps_g = ctx.enter_context(tc.tile_pool(name="ps_g", bufs=6, space="PSUM"))

        def wslice(w, i, g):
            return w[:, (i * 3 + g) * C:(i * 3 + g + 1) * C]

        def bslice(i, k):
            return bias[:, i * 4 + k:i * 4 + k + 1]

        def nm3(t):  # [P, N] tile viewed as node-major [P, NT, C]-style 3D
            return t[:].rearrange("p (a b) -> p a b", b=P)

        loop_cm = (tc.For_i(0, loop_reps, 1, hint_engines=(mybir.EngineType.PE,))
                   if loop_reps > 1 else nullcontext())
        with loop_cm:
          for _rep in range(reps):
            # ---- input loads -------------------------------------------------
            # x + weights ride the ACT HWDGE ring (x first — needed first);
            # adjacency rides the SP HWDGE ring; the SP ring later carries the
            # xbar transposes + stores.
            adj_sb = []
            x_sb = []
            for b in range(BPC):
                xt = xo_pool.tile([P, NT, C], F32, tag=f"xo{b}")
                nc.scalar.dma_start(xt[:], x_d[b].rearrange("(t p) c -> p t c", p=P))
                x_sb.append(xt)
                a = adj_pool.tile([P, NT, N], F16, tag=f"adj{b}")
                # adj was cast to fp16 on the host (exact for 0/1 entries).
                # Two chunks so the first j-tiles land early; j = t*128+p.
                src = adj_d[b].rearrange("(t p) n -> p t n", p=P)
                nc.sync.dma_start(a[:, 0:NT // 2, :], src[:, 0:NT // 2, :])
                nc.sync.dma_start(a[:, NT // 2:, :], src[:, NT // 2:, :])
                adj_sb.append(a)
            wch = consts.tile([P, L * 3 * C], F16)
            nc.scalar.dma_start(wch[:], wch_d.rearrange("c l g d -> c (l g d)"))
            wcl = consts.tile([P, L * 3 * C], F16)
            nc.scalar.dma_start(wcl[:], wcl_d.rearrange("c l g d -> c (l g d)"))
            whh = consts.tile([P, L * 3 * C], F16)
            nc.scalar.dma_start(whh[:], whh_d.rearrange("c l g d -> c (l g d)"))
            bias = consts.tile([P, L * 4], F32)
            nc.scalar.dma_start(bias[:], bias_d.rearrange("c l k -> c (l k)"))
            identity = consts.tile([P, P], F32)
            make_identity(nc, identity)

            # ---- layer-0 state init (h0 = x) --------------------------------
            # The feature-major copy of x is built with PE transposes: the PE
            # is idle during the load phase anyway, and this keeps the SP DMA
            # ring free for the adjacency loads.
            h_nm_hi = [None] * BPC
            h_nm_lo = [None] * BPC
            hT = [None] * BPC
            hT_hi = [None] * BPC
            for b in range(BPC):
                xt = x_sb[b]
                hi = hnm_pool.tile([P, NT, C], F16, tag=f"hnmh{b}")
                lo = hnm_pool.tile([P, NT, C], F16, tag=f"hnml{b}")
                nc.scalar.activation(hi[:], xt[:], AF.Copy)
                nc.gpsimd.tensor_sub(lo[:], xt[:], hi[:])
                h = hT_pool.tile([P, N], F32, tag=f"hT{b}")
                hh = hTh_pool.tile([P, N], F16, tag=f"hTh{b}")
                for k in range(2):
                    ps = ps_g.tile([P, HALF], F32, tag="psg")
                    for j in range(4):
                        nc.tensor.transpose(ps[:, j * P:(j + 1) * P],
                                            xt[:, k * 4 + j, :], identity[:])
                    nc.any.tensor_copy(h[:, k * HALF:(k + 1) * HALF], ps[:])
                    nc.any.tensor_copy(hh[:, k * HALF:(k + 1) * HALF], ps[:])
                h_nm_hi[b], h_nm_lo[b], hT[b], hT_hi[b] = hi, lo, h, hh

            # ---- layers ------------------------------------------------------
            for i in range(L):
                # h_lo passes through A^T only in the first S_RF layers
                # (perturbations introduced late amplify far less);
                # fp16 state once the next layer no longer needs h_lo.
                s_split = i < S_RF
                last_layer = i == L - 1
                state_f16 = i + 1 >= S_RF
                produce_lo = (i + 1 < S_RF) or (last_layer and not state_f16)
                for b in range(BPC):
                    # sT = (A^T (h_hi [+ h_lo]))^T accumulated in psum, fp32
                    s_hi = sT_pool.tile([P, N], F16, tag="shi")
                    s_lo = sT_pool.tile([P, N], F16, tag="slo")
                    stats = (h_nm_hi[b], h_nm_lo[b]) if s_split else (h_nm_hi[b],)
                    for half in range(2):
                        hs = slice(half * HALF, (half + 1) * HALF)
                        ps = ps_s.tile([P, HALF], F32, tag="ps_s")
                        for ti, hnm in enumerate(stats):
                            for j in range(NT):
                                nc.tensor.matmul(
                                    ps[:],
                                    lhsT=hnm[:, j, :],
                                    rhs=adj_sb[b][:, j, hs],
                                    start=(ti == 0 and j == 0),
                                    stop=(ti == len(stats) - 1 and j == NT - 1),
                                )
                        nc.scalar.activation(s_hi[:, hs], ps[:], AF.Copy)
                        nc.vector.tensor_sub(s_lo[:, hs], ps[:], s_hi[:, hs])

                    if state_f16:
                        new_h = hTh_pool.tile([P, N], F16, tag=f"hTh{b}")
                        new_hh = new_h
                    else:
                        new_h = hT_pool.tile([P, N], F32, tag=f"hT{b}")
                        new_hh = hTh_pool.tile([P, N], F16, tag=f"hTh{b}")
                    if produce_lo:
                        new_hl = hTl_pool.tile([P, N], F16, tag=f"hTl{b}")
                    if last_layer and state_f16:
                        yh = hnm_pool.tile([P, NT, C], F16, tag=f"hnmh{b}")
                        ost = xo_pool.tile([P, NT, C], F32, tag=f"xo{b}")

                    for nh in range(2):
                        sl = slice(nh * HALF, (nh + 1) * HALF)
                        pr = ps_g.tile([P, HALF], F32, tag="psg")
                        pz = ps_g.tile([P, HALF], F32, tag="psg")
                        pxn = ps_g.tile([P, HALF], F32, tag="psg")
                        phn = ps_g.tile([P, HALF], F32, tag="psg")
                        for g, pg in ((0, pr), (1, pz)):
                            nc.tensor.matmul(pg[:], lhsT=wslice(wch, i, g),
                                             rhs=s_hi[:, sl], start=True, stop=False)
                            nc.tensor.matmul(pg[:], lhsT=wslice(wch, i, g),
                                             rhs=s_lo[:, sl], start=False, stop=False)
                            nc.tensor.matmul(pg[:], lhsT=wslice(wcl, i, g),
                                             rhs=s_hi[:, sl], start=False, stop=False)
                            nc.tensor.matmul(pg[:], lhsT=wslice(whh, i, g),
                                             rhs=hT_hi[b][:, sl], start=False, stop=True)
                        nc.tensor.matmul(pxn[:], lhsT=wslice(wch, i, 2),
                                         rhs=s_hi[:, sl], start=True, stop=False)
                        nc.tensor.matmul(pxn[:], lhsT=wslice(wch, i, 2),
                                         rhs=s_lo[:, sl], start=False, stop=False)
                        nc.tensor.matmul(pxn[:], lhsT=wslice(wcl, i, 2),
                                         rhs=s_hi[:, sl], start=False, stop=True)
                        nc.tensor.matmul(phn[:], lhsT=wslice(whh, i, 2),
                                         rhs=hT_hi[b][:, sl], start=True, stop=True)

                        r = ew_pool.tile([P, HALF], F32, tag="ew")
                        nc.scalar.activation(r[:], pr[:], AF.Sigmoid, bias=bslice(i, 0))
                        zc = ew_pool.tile([P, HALF], F32, tag="ew")
                        nc.scalar.activation(zc[:], pz[:], AF.Sigmoid,
                                             bias=bslice(i, 1), scale=-1.0)
                        t = ew_pool.tile([P, HALF], F32, tag="ew")
                        nc.vector.scalar_tensor_tensor(t[:], phn[:], bslice(i, 3), r[:],
                                                       op0=ALU.add, op1=ALU.mult)
                        u = ew_pool.tile([P, HALF], F32, tag="ew")
                        nc.vector.scalar_tensor_tensor(u[:], pxn[:], bslice(i, 2), t[:],
                                                       op0=ALU.add, op1=ALU.add)
                        nt = ew_pool.tile([P, HALF], F32, tag="ew")
                        nc.scalar.activation(nt[:], u[:], AF.Tanh)
                        d = ew_pool.tile([P, HALF], F32, tag="ew")
                        nc.gpsimd.tensor_sub(d[:], nt[:], hT[b][:, sl])
                        e = ew_pool.tile([P, HALF], F32, tag="ew")
                        nc.gpsimd.tensor_mul(e[:], zc[:], d[:])
                        nc.vector.tensor_add(new_h[:, sl], hT[b][:, sl], e[:])
                        if not state_f16:
                            nc.scalar.activation(new_hh[:, sl], new_h[:, sl], AF.Copy)
                            if produce_lo:
                                nc.gpsimd.tensor_sub(new_hl[:, sl], new_h[:, sl],
                                                     new_hh[:, sl])
                        if last_layer and state_f16:
                            # stream the output out per half: transpose to
                            # node-major, widen to fp32, store
                            ht = slice(nh * (NT // 2), (nh + 1) * (NT // 2))
                            nc.sync.dma_start(out=yh[:, ht, :], in_=new_h[:, sl],
                                              transpose=True)
                            nc.any.tensor_copy(ost[:, ht, :], yh[:, ht, :])
                            nc.sync.dma_start(
                                out=y_d[b].rearrange("(t p) c -> p t c", p=P)[:, ht, :],
                                in_=ost[:, ht, :])

                    hT[b] = new_h
                    hT_hi[b] = new_hh
                    if not last_layer:
                        nhi = hnm_pool.tile([P, NT, C], F16, tag=f"hnmh{b}")
                        nc.sync.dma_start(out=nhi[:], in_=new_hh[:], transpose=True)
                        h_nm_hi[b] = nhi
                        if produce_lo:
                            nlo = hnm_pool.tile([P, NT, C], F16, tag=f"hnml{b}")
                            nc.sync.dma_start(out=nlo[:], in_=new_hl[:], transpose=True)
                            h_nm_lo[b] = nlo
                    elif not state_f16:
                        ost = xo_pool.tile([P, NT, C], F32, tag=f"xo{b}")
                        yh = hnm_pool.tile([P, NT, C], F16, tag=f"hnmh{b}")
                        yl = hnm_pool.tile([P, NT, C], F16, tag=f"hnml{b}")
                        nc.sync.dma_start(out=yh[:], in_=new_hh[:], transpose=True)
                        nc.sync.dma_start(out=yl[:], in_=new_hl[:], transpose=True)
                        nc.vector.tensor_add(ost[:], yh[:], yl[:])
                        nc.sync.dma_start(
                            out=y_d[b].rearrange("(t p) c -> p t c", p=P), in_=ost[:])

    nc.finalize()
    return nc


def _prep_weights(weight, w_ih, w_hh, b_ih, b_hh):
    weight = np.asarray(weight, np.float32)
    w_ih = np.asarray(w_ih, np.float32)
    w_hh = np.asarray(w_hh, np.float32)
    b_ih = np.asarray(b_ih, np.float32)
    b_hh = np.asarray(b_hh, np.float32)

    # fused input-gate weight: gx = s @ (W @ w_ih^T), as [C, L, 3, C]
    wc = np.einsum("lcd,lgd->lcg", weight, w_ih)          # [L, C, 3C]
    wch = wc.astype(np.float16)
    wcl = (wc - wch.astype(np.float32)).astype(np.float16)
    whh_t = np.transpose(w_hh, (0, 2, 1)).astype(np.float16)  # [L, C, 3C]

    def to_clgd(a):  # [L, C, 3C] -> [C, L, 3, C]
        return np.ascontiguousarray(
            np.transpose(a.reshape(L, C, 3, C), (1, 0, 2, 3)))

    bias = np.empty((C, L, 4), np.float32)
    bias[:, :, 0] = (b_ih[:, 0:C] + b_hh[:, 0:C]).T
    bias[:, :, 1] = -(b_ih[:, C:2 * C] + b_hh[:, C:2 * C]).T
    bias[:, :, 2] = b_ih[:, 2 * C:3 * C].T
    bias[:, :, 3] = b_hh[:, 2 * C:3 * C].T

    return to_clgd(wch), to_clgd(wcl), to_clgd(whh_t), bias


def kernel(x, adj, mask, weight, w_ih, w_hh, b_ih, b_hh, _run_kwargs=None):
    x = np.asarray(x, np.float32)
    # binary adjacency: fp16 is exact, halves the HBM traffic on device
    adj = np.asarray(adj, np.float32).astype(np.float16)
    mask = np.asarray(mask, np.float32)
    wch, wcl, whh, bias = _prep_weights(weight, w_ih, w_hh, b_ih, b_hh)

    if "nc" not in _PROGRAM_CACHE:
        _PROGRAM_CACHE["nc"] = _build_program()
    nc = _PROGRAM_CACHE["nc"]

    in_maps = []
    for c in range(NCORES):
        sl = slice(c * BPC, (c + 1) * BPC)
        in_maps.append({
            "x": np.ascontiguousarray(x[sl]),
            "adj": np.ascontiguousarray(adj[sl]),
            "wch": wch, "wcl": wcl, "whh": whh, "bias": bias,
        })

    res = run_bass_kernel_spmd(nc, in_maps, list(range(NCORES)),
                               **(_run_kwargs or {}))
    y = np.concatenate([r["y"] for r in res.results], axis=0)
    y = y * mask[:, :, None]
    if _run_kwargs:
        kernel.last_results = res
    return y.astype(np.float32)

